# revision 1
# baseline (speedup 1.0000x reference)
"""Sparse attention (talking-heads + top-64) Trainium2 kernel, 8-core SPMD.

Sharding: (batch, query-block) across 8 cores. Core c handles batch c//4;
its batch-group index g=c%4 selects query-blocks [g, 7-g, 8+g, 15-g], one
per "slot"; slot s is compiled for the max J of its qb group so the SPMD
program is identical on every core.

Column grid per row: [16 mem keys | 112 pad | 2048 seq keys], slot widths
J = 128 + 512*(s+1) = [640, 1152, 1664, 2176].

Per slot: QK (split-bf16 3-term, per head) -> ACT evac -> DMA partition
shuffle to (i_sub, h) rows -> block-diag pre-mix matmul (fp32) + additive
causal/pad mask matmuls -> ACT evac -> DRAM dm buffer. Per row-group g:
8-round max8/match_replace top-64 (DVE) -> denominator from the 64
extracted values -> ACT exp with folded -max-ln(den) bias -> DVE is_ge
mask + GP multiply -> p~ bf16 -> fused post-mix+transpose matmul (bf16)
-> pT -> AV (bf16) -> Wo projection (bf16) + bias -> out.
"""
import numpy as np
import ml_dtypes

bf16 = ml_dtypes.bfloat16

B, N, DIM = 2, 2048, 1024
H, D, M = 16, 64, 16
TOPK = 64
NEG = -1.0e30
SLOT_J = [640, 1152, 1664, 2176]
NSLOT = 4
CW = 256  # QK/mix j-chunk width
QBS_OF_G = [[g, 7 - g, 8 + g, 15 - g] for g in range(4)]

_nc_cache = {}


def _split_hi_lo(a):
    hi = a.astype(bf16)
    lo = (a.astype(np.float32) - hi.astype(np.float32)).astype(bf16)
    return hi, lo


def _chunk_part(a):
    """[K, F] -> [128, K//128, F] with partition = K % 128 within chunk."""
    K, F = a.shape
    return np.ascontiguousarray(a.reshape(K // 128, 128, F).transpose(1, 0, 2))


def build_nc(reps=1):
    import concourse.bass as bass
    import concourse.tile as tile
    import concourse.mybir as mybir
    from concourse import bacc

    dt = mybir.dt
    nc = bacc.Bacc()

    def din(name, shape, d=dt.bfloat16):
        return nc.dram_tensor(name, shape, d, kind="ExternalInput")

    xT_hi = din("xT_hi", [128, 8, N])
    xT_lo = din("xT_lo", [128, 8, N])
    xqT_hi = din("xqT_hi", [128, 8, NSLOT, 128])
    xqT_lo = din("xqT_lo", [128, 8, NSLOT, 128])
    wq_hi = din("wq_hi", [128, 8, DIM])
    wq_lo = din("wq_lo", [128, 8, DIM])
    wk_hi = din("wk_hi", [128, 8, DIM])
    wk_lo = din("wk_lo", [128, 8, DIM])
    wv = din("wv", [128, 8, DIM])
    wo = din("wo", [128, 8, DIM])
    memKT_hi = din("memKT_hi", [128, 8, 128])
    memKT_lo = din("memKT_lo", [128, 8, 128])
    memV = din("memV", [128, DIM])
    w1 = din("w1", [128, 128], dt.float32)
    w2 = din("w2", [128, 128])
    selg = din("selg", [128, 16, 128], dt.float32)
    ones1 = din("ones1", [1, 128], dt.float32)
    padrow = din("padrow", [1, CW], dt.float32)
    masks = din("masks", [128, NSLOT, 640], dt.float32)
    bo_in = din("bo_in", [128, DIM], dt.float32)

    out_d = nc.dram_tensor("out", [NSLOT, 128, DIM], dt.float32,
                           kind="ExternalOutput")

    with tile.TileContext(nc) as tc:
      for _rep in range(reps):
        with tc.tile_pool(name="persist", bufs=1) as pool_w, \
             tc.tile_pool(name="dscratch", bufs=1, space="DRAM") as pool_dram, \
             tc.tile_pool(name="dots", bufs=1) as pool_dots, \
             tc.tile_pool(name="kts", bufs=2) as pool_kts, \
             tc.tile_pool(name="shuf", bufs=3) as pool_shuf, \
             tc.tile_pool(name="dmc", bufs=3) as pool_dmc, \
             tc.tile_pool(name="psD", bufs=2, space="PSUM") as pool_psd, \
             tc.tile_pool(name="psMix", bufs=2, space="PSUM") as pool_psmx:

            wo_s = pool_w.tile([128, 8, DIM], dt.bfloat16)
            w1_s = pool_w.tile([128, 128], dt.float32)
            w2_s = pool_w.tile([128, 128], dt.bfloat16)
            selg_s = pool_w.tile([128, 16, 128], dt.float32)
            ones1_s = pool_w.tile([1, 128], dt.float32)
            padrow_s = pool_w.tile([1, CW], dt.float32)
            masks_s = pool_w.tile([128, NSLOT, 640], dt.float32)
            bo_s = pool_w.tile([128, DIM], dt.float32)
            qt_hi = pool_w.tile([128, NSLOT, 8, 128], dt.bfloat16)
            qt_lo = pool_w.tile([128, NSLOT, 8, 128], dt.bfloat16)
            for c in range(8):
                nc.sync.dma_start(wo_s[:, c, :], wo[:, c, :])
            nc.sync.dma_start(w1_s[:], w1[:])
            nc.sync.dma_start(w2_s[:], w2[:])
            nc.sync.dma_start(selg_s[:], selg[:])
            nc.sync.dma_start(ones1_s[:], ones1[:])
            nc.sync.dma_start(padrow_s[:], padrow[:])
            nc.sync.dma_start(masks_s[:], masks[:])
            nc.sync.dma_start(bo_s[:], bo_in[:])

            kt_hi_d = pool_dram.tile([128, 8, 2176], dt.bfloat16)
            kt_lo_d = pool_dram.tile([128, 8, 2176], dt.bfloat16)
            v_d = pool_dram.tile([128, 16, 17, 64], dt.bfloat16)

            def do_jc(s, J, NJC, NJP, dm_dram, jc):
                W = CW if jc < NJC - 1 else 128
                j0 = jc * CW
                kth = pool_kts.tile([128, 8, CW], dt.bfloat16, tag="kth")
                ktl = pool_kts.tile([128, 8, CW], dt.bfloat16, tag="ktl")
                nc.sync.dma_start(kth[:, :, 0:W], kt_hi_d[:, :, j0:j0 + W])
                nc.sync.dma_start(ktl[:, :, 0:W], kt_lo_d[:, :, j0:j0 + W])
                dots = pool_dots.tile([128, 16, CW], dt.float32, tag="dots")
                for h in range(16):
                    pb = (h % 2) * 64
                    pl = h // 2
                    psd = pool_psd.tile([128, CW], dt.float32, tag="psd")
                    lhi = qt_hi[pb:pb + 64, s, pl, :]
                    llo = qt_lo[pb:pb + 64, s, pl, :]
                    rhi = kth[pb:pb + 64, pl, 0:W]
                    rlo = ktl[pb:pb + 64, pl, 0:W]
                    nc.tensor.matmul(psd[:, 0:W], lhi, rhi, start=True,
                                     stop=False)
                    nc.tensor.matmul(psd[:, 0:W], lhi, rlo, start=False,
                                     stop=False)
                    nc.tensor.matmul(psd[:, 0:W], llo, rhi, start=False,
                                     stop=True)
                    nc.scalar.copy(dots[:, h, 0:W], psd[:, 0:W])
                mwin0 = J - 640
                dmc = None
                for g in range(16):
                    shuf = pool_shuf.tile([128, CW], dt.float32, tag="shuf")
                    nc.gpsimd.dma_start(
                        shuf[:, 0:W], dots[g * 8:(g + 1) * 8, :, 0:W])
                    psm = pool_psmx.tile([128, CW], dt.float32, tag="psm")
                    mm = [(w1_s[:], shuf[:, 0:W], slice(0, W))]
                    if jc == 0:
                        mm.append((ones1_s[:], padrow_s[:, 0:W], slice(0, W)))
                    if j0 + W > mwin0:
                        a = max(j0, mwin0)
                        mm.append((selg_s[:, g, :],
                                   masks_s[:, s, a - mwin0:j0 + W - mwin0],
                                   slice(a - j0, W)))
                    for mi, (lh, rh, csl) in enumerate(mm):
                        nc.tensor.matmul(psm[:, csl], lh, rh,
                                         start=(mi == 0),
                                         stop=(mi == len(mm) - 1))
                    if g % 4 == 0:
                        dmc = pool_dmc.tile([128, 4, CW], dt.float32,
                                            tag="dmc")
                    nc.scalar.copy(dmc[:, g % 4, 0:W], psm[:, 0:W])
                    if g % 4 == 3:
                        nc.sync.dma_start(
                            dm_dram[g - 3:g + 1, :, j0:j0 + W]
                            .rearrange("g p j -> p g j"), dmc[:, :, 0:W])

            dm_s0 = pool_dram.tile([16, 128, 2176], dt.float32, tag="dm0")

            # ============ phase 1: QT, K^T (slot-0 jc interleaved), V ======
            with tc.tile_pool(name="ph1", bufs=1) as p1, \
                 tc.tile_pool(name="ph1c", bufs=3) as p1c, \
                 tc.tile_pool(name="ph1x", bufs=2) as p1x, \
                 tc.tile_pool(name="ph1q", bufs=2) as p1q, \
                 tc.tile_pool(name="ps1", bufs=2, space="PSUM") as ps1:
                wv_s = p1.tile([128, 8, DIM], dt.bfloat16)
                xq_hi_s = p1.tile([128, 8, NSLOT, 128], dt.bfloat16)
                xq_lo_s = p1.tile([128, 8, NSLOT, 128], dt.bfloat16)
                for c in range(8):
                    nc.sync.dma_start(wv_s[:, c, :], wv[:, c, :])
                nc.sync.dma_start(xq_hi_s[:], xqT_hi[:])
                nc.sync.dma_start(xq_lo_s[:], xqT_lo[:])
                for c in range(8):
                    nc.sync.dma_start(kt_hi_d[:, c, 0:128], memKT_hi[:, c, :])
                    nc.sync.dma_start(kt_lo_d[:, c, 0:128], memKT_lo[:, c, :])
                nc.sync.dma_start(v_d[:, :, 0, :], memV[:])

                # QT (wq streamed per m-chunk)
                for mc in range(8):
                    wqc_h = p1q.tile([128, 8, 128], dt.bfloat16, tag="wqch")
                    wqc_l = p1q.tile([128, 8, 128], dt.bfloat16, tag="wqcl")
                    nc.sync.dma_start(wqc_h[:],
                                      wq_hi[:, :, mc * 128:(mc + 1) * 128])
                    nc.sync.dma_start(wqc_l[:],
                                      wq_lo[:, :, mc * 128:(mc + 1) * 128])
                    for s in range(NSLOT):
                        psq = ps1.tile([128, 128], dt.float32, tag="psq")
                        for dc in range(8):
                            nc.tensor.matmul(psq[:], wqc_h[:, dc, :],
                                             xq_hi_s[:, dc, s, :],
                                             start=(dc == 0), stop=False)
                            nc.tensor.matmul(psq[:], wqc_h[:, dc, :],
                                             xq_lo_s[:, dc, s, :],
                                             start=False, stop=False)
                            nc.tensor.matmul(psq[:], wqc_l[:, dc, :],
                                             xq_hi_s[:, dc, s, :],
                                             start=False, stop=(dc == 7))
                        qt32 = p1q.tile([128, 128], dt.float32, tag="qt32")
                        nc.scalar.copy(qt32[:], psq[:])
                        nc.vector.tensor_copy(qt_hi[:, s, mc, :], qt32[:])
                        nc.vector.tensor_sub(qt_lo[:, s, mc, :], qt32[:],
                                             qt_hi[:, s, mc, :])

                def do_kt(njc):
                    xth = p1x.tile([128, 8, 512], dt.bfloat16, tag="xth")
                    xtl = p1x.tile([128, 8, 512], dt.bfloat16, tag="xtl")
                    nc.sync.dma_start(
                        xth[:], xT_hi[:, :, njc * 512:(njc + 1) * 512])
                    nc.sync.dma_start(
                        xtl[:], xT_lo[:, :, njc * 512:(njc + 1) * 512])
                    for ic in range(8):
                        wkc_h = p1q.tile([128, 8, 128], dt.bfloat16,
                                         tag="wkch")
                        wkc_l = p1q.tile([128, 8, 128], dt.bfloat16,
                                         tag="wkcl")
                        nc.sync.dma_start(
                            wkc_h[:], wk_hi[:, :, ic * 128:(ic + 1) * 128])
                        nc.sync.dma_start(
                            wkc_l[:], wk_lo[:, :, ic * 128:(ic + 1) * 128])
                        ps = ps1.tile([128, 512], dt.float32, tag="psk")
                        for dc in range(8):
                            nc.tensor.matmul(ps[:], wkc_h[:, dc, :],
                                             xth[:, dc, :], start=(dc == 0),
                                             stop=False)
                            nc.tensor.matmul(ps[:], wkc_h[:, dc, :],
                                             xtl[:, dc, :], start=False,
                                             stop=False)
                            nc.tensor.matmul(ps[:], wkc_l[:, dc, :],
                                             xth[:, dc, :], start=False,
                                             stop=(dc == 7))
                        khi = p1c.tile([128, 512], dt.bfloat16, tag="khi")
                        klo = p1c.tile([128, 512], dt.bfloat16, tag="klo")
                        nc.scalar.copy(khi[:], ps[:])
                        nc.vector.tensor_sub(klo[:], ps[:], khi[:])
                        cols = slice(128 + njc * 512, 128 + (njc + 1) * 512)
                        nc.sync.dma_start(kt_hi_d[:, ic, cols], khi[:])
                        nc.sync.dma_start(kt_lo_d[:, ic, cols], klo[:])

                do_kt(0)
                # slot-0 QK/mix pipeline interleaves with remaining phase-1
                for jc in range(3):
                    do_jc(0, SLOT_J[0], 3, 5, dm_s0, jc)
                for njc in range(1, 4):
                    do_kt(njc)

                for nb in range(16):
                    xthv = p1x.tile([128, 8, 128], dt.bfloat16, tag="xthv")
                    nc.sync.dma_start(
                        xthv[:], xT_hi[:, :, nb * 128:(nb + 1) * 128])
                    for fh in range(2):
                        ps = ps1.tile([128, 512], dt.float32, tag="psk")
                        for dc in range(8):
                            nc.tensor.matmul(
                                ps[:], xthv[:, dc, :],
                                wv_s[:, dc, fh * 512:(fh + 1) * 512],
                                start=(dc == 0), stop=(dc == 7))
                        vsb = p1c.tile([128, 512], dt.bfloat16, tag="vsb")
                        nc.scalar.copy(vsb[:], ps[:])
                        nc.sync.dma_start(
                            v_d[:, fh * 8:(fh + 1) * 8, 1 + nb, :], vsb[:])

            # ============ phase 2: attention =============================
            with tc.tile_pool(name="dmg", bufs=2) as pool_dmg, \
                 tc.tile_pool(name="tk1", bufs=1) as pool_tk1, \
                 tc.tile_pool(name="tk2", bufs=2) as pool_tk2, \
                 tc.tile_pool(name="sm", bufs=4) as pool_sm, \
                 tc.tile_pool(name="ptq", bufs=1) as pool_ptq, \
                 tc.tile_pool(name="vs", bufs=2) as pool_vs, \
                 tc.tile_pool(name="outs", bufs=1) as pool_out, \
                 tc.tile_pool(name="psPt", bufs=2, space="PSUM") as pool_pspt, \
                 tc.tile_pool(name="psAvO", bufs=2, space="PSUM") as pool_psav:

                def do_gq(s, J, NJC, NJP, dm_dram, av32, gq):
                    ptq = pool_ptq.tile([128, 17, 16, 32], dt.bfloat16,
                                        tag="ptq")
                    for gi in range(4):
                        g = gq * 4 + gi
                        dmg = pool_dmg.tile([128, 2176], dt.float32,
                                            tag="dmg")
                        nc.sync.dma_start(dmg[:, 0:J], dm_dram[g, :, 0:J])
                        scr = pool_tk1.tile([128, 2176], dt.float32,
                                            tag="scr")
                        cands = pool_tk2.tile([128, 64], dt.float32,
                                              tag="cands")
                        if s == 0:
                            nc.vector.max(cands[:, 0:8], dmg[:, 0:J])
                            nc.vector.match_replace(scr[:, 0:J],
                                                    cands[:, 0:8],
                                                    dmg[:, 0:J], NEG)
                            for r in range(1, 8):
                                nc.vector.max(cands[:, r * 8:(r + 1) * 8],
                                              scr[:, 0:J])
                                if r < 7:
                                    nc.vector.match_replace(
                                        scr[:, 0:J],
                                        cands[:, r * 8:(r + 1) * 8],
                                        scr[:, 0:J], NEG)
                        else:
                            RR = 4 if s == 1 else 3
                            segw = (J - 128) // 8
                            segs = [(0, 128)] + [
                                (128 + k * segw, 128 + (k + 1) * segw)
                                for k in range(8)]
                            cpool = pool_tk2.tile([128, 9 * 8 * 4],
                                                  dt.float32, tag="cpool")
                            for rr in range(RR):
                                src_t = dmg if rr == 0 else scr
                                for si2, (a2, b2) in enumerate(segs):
                                    nc.vector.max(
                                        cpool[:, (rr * 9 + si2) * 8:
                                              (rr * 9 + si2) * 8 + 8],
                                        src_t[:, a2:b2])
                                if rr < RR - 1:
                                    for si2, (a2, b2) in enumerate(segs):
                                        nc.vector.match_replace(
                                            scr[:, a2:b2],
                                            cpool[:, (rr * 9 + si2) * 8:
                                                  (rr * 9 + si2) * 8 + 8],
                                            src_t[:, a2:b2], NEG)
                            ncand = 9 * 8 * RR
                            nc.vector.max(cands[:, 0:8], cpool[:, 0:ncand])
                            nc.vector.match_replace(cpool[:, 0:ncand],
                                                    cands[:, 0:8],
                                                    cpool[:, 0:ncand], NEG)
                            for r in range(1, 8):
                                nc.vector.max(cands[:, r * 8:(r + 1) * 8],
                                              cpool[:, 0:ncand])
                                if r < 7:
                                    nc.vector.match_replace(
                                        cpool[:, 0:ncand],
                                        cands[:, r * 8:(r + 1) * 8],
                                        cpool[:, 0:ncand], NEG)
                        negm = pool_sm.tile([128, 1], dt.float32, tag="negm")
                        nc.vector.tensor_scalar_mul(negm[:], cands[:, 0:1],
                                                    -1.0)
                        ec = pool_sm.tile([128, 64], dt.float32, tag="ec")
                        nc.scalar.activation(ec[:], cands[:],
                                             mybir.ActivationFunctionType.Exp,
                                             bias=negm[:])
                        den = pool_sm.tile([128, 1], dt.float32, tag="den")
                        nc.vector.reduce_sum(den[:], ec[:],
                                             axis=mybir.AxisListType.X)
                        rden = pool_sm.tile([128, 1], dt.float32, tag="rden")
                        nc.vector.reciprocal(rden[:], den[:])
                        p2 = pool_tk2.tile([128, 2176], dt.float32, tag="p2")
                        nc.scalar.activation(p2[:, 0:J], dmg[:, 0:J],
                                             mybir.ActivationFunctionType.Exp,
                                             bias=negm[:])
                        nc.vector.tensor_scalar(scr[:, 0:J], dmg[:, 0:J],
                                                cands[:, 63:64], rden[:],
                                                mybir.AluOpType.is_ge,
                                                mybir.AluOpType.mult)
                        ptil = pool_tk2.tile([128, 2176], dt.bfloat16,
                                             tag="ptil")
                        nc.gpsimd.tensor_mul(ptil[:, 0:J], scr[:, 0:J],
                                             p2[:, 0:J])
                        for jp4 in range((NJP + 3) // 4):
                            nq = min(4, NJP - jp4 * 4)
                            pspt = pool_pspt.tile([128, 512], dt.float32,
                                                  tag="pspt")
                            for q in range(nq):
                                jp = jp4 * 4 + q
                                nc.tensor.matmul(
                                    pspt[:, q * 128:(q + 1) * 128],
                                    ptil[:, jp * 128:(jp + 1) * 128],
                                    w2_s[:], start=True, stop=True)
                            dst = ptq[:, jp4 * 4:jp4 * 4 + nq, :,
                                      gi * 8:(gi + 1) * 8]
                            nc.scalar.copy(dst[:], pspt[:, 0:nq * 128])
                    # AV over this quad (32 query columns)
                    psav = pool_psav.tile([128, 8, 32], dt.float32,
                                          tag="avo")
                    for ko in range(16):
                        vst = pool_vs.tile([128, 17, 64], dt.bfloat16,
                                           tag="vst")
                        nc.gpsimd.dma_start(vst[:, 0:NJP, :],
                                            v_d[:, ko, 0:NJP, :])
                        pb = (ko % 2) * 64
                        for jp in range(NJP):
                            nc.tensor.matmul(
                                psav[pb:pb + 64, ko // 2, :],
                                vst[:, jp, :], ptq[:, jp, ko, :],
                                start=(jp == 0), stop=(jp == NJP - 1))
                    nc.vector.tensor_copy(av32[:, :, gq * 32:(gq + 1) * 32],
                                          psav[:])

                def do_tail(s, J, av32):
                    av_sb = pool_out.tile([128, 8, 128], dt.bfloat16,
                                          tag="av")
                    nc.vector.tensor_copy(av_sb[:], av32[:])
                    osb = pool_out.tile([128, DIM], dt.float32, tag="osb")
                    for fh in range(2):
                        pso = pool_psav.tile([128, 512], dt.float32,
                                             tag="avo")
                        for cp in range(8):
                            nc.tensor.matmul(
                                pso[:], av_sb[:, cp, :],
                                wo_s[:, cp, fh * 512:(fh + 1) * 512],
                                start=(cp == 0), stop=(cp == 7))
                        nc.vector.tensor_add(osb[:, fh * 512:(fh + 1) * 512],
                                             pso[:],
                                             bo_s[:, fh * 512:(fh + 1) * 512])
                    nc.sync.dma_start(out_d[s, :, :], osb[:])

                for s in range(NSLOT):
                    J = SLOT_J[s]
                    NJC = (J - 128) // CW + 1
                    NJP = J // 128
                    if s == 0:
                        dm_dram = dm_s0
                    else:
                        dm_dram = pool_dram.tile([16, 128, 2176], dt.float32,
                                                 tag=f"dm{s % 2}")
                        for jc in range(NJC):
                            do_jc(s, J, NJC, NJP, dm_dram, jc)
                    av32 = pool_out.tile([128, 8, 128], dt.float32,
                                         tag="av32")
                    for gq in range(4):
                        do_gq(s, J, NJC, NJP, dm_dram, av32, gq)
                    do_tail(s, J, av32)

    nc.finalize()
    return nc


_prep_cache = {}


def _host_prep(core, inputs):
    x = np.asarray(inputs["x"], dtype=np.float32)
    Wq = np.asarray(inputs["Wq"], dtype=np.float32) * (D ** -0.5)
    Wk = np.asarray(inputs["Wk"], dtype=np.float32)
    Wv = np.asarray(inputs["Wv"], dtype=np.float32)
    Wo = np.asarray(inputs["Wo"], dtype=np.float32)
    bo = np.asarray(inputs["bo"], dtype=np.float32)
    pre = np.asarray(inputs["pre_proj"], dtype=np.float32)
    post = np.asarray(inputs["post_proj"], dtype=np.float32)
    mem_k = np.asarray(inputs["mem_k"], dtype=np.float32)
    mem_v = np.asarray(inputs["mem_v"], dtype=np.float32)

    b = core // 4
    g = core % 4
    qbs = QBS_OF_G[g]

    xb = x[b]
    if ("xT", b) not in _prep_cache:
        xT = np.ascontiguousarray(xb.T)
        _prep_cache[("xT", b)] = _split_hi_lo(xT)
    xT_hi, xT_lo = _prep_cache[("xT", b)]
    xq = np.concatenate([xb[qb * 128:(qb + 1) * 128] for qb in qbs], axis=0)
    xqT = np.ascontiguousarray(xq.T)                      # [DIM, 512]
    xqT_hi, xqT_lo = _split_hi_lo(xqT)

    if "w" not in _prep_cache:
        _prep_cache["w"] = (_split_hi_lo(Wq), _split_hi_lo(Wk))
    (wq_hi, wq_lo), (wk_hi, wk_lo) = _prep_cache["w"]

    mkt = np.zeros((128, 8, 128), dtype=np.float32)
    for h in range(H):
        mkt[(h % 2) * 64:(h % 2) * 64 + 64, h // 2, 0:M] = mem_k[h].T
    mkt_hi, mkt_lo = _split_hi_lo(mkt)
    mv = np.zeros((128, DIM), dtype=np.float32)
    mv[0:M] = mem_v.transpose(1, 0, 2).reshape(M, DIM)

    w1 = np.zeros((128, 128), dtype=np.float32)
    for isub in range(8):
        for h in range(H):
            for k in range(H):
                w1[isub * 16 + h, k * 8 + isub] = pre[h, k]
    w2 = np.zeros((128, 128), dtype=np.float32)
    for isub in range(8):
        for k in range(H):
            for ko in range(H):
                w2[k * 8 + isub, ko * 8 + isub] = post[k, ko]
    selg = np.zeros((128, 16, 128), dtype=np.float32)
    for gg in range(16):
        for isub in range(8):
            selg[gg * 8 + isub, gg, np.arange(16) * 8 + isub] = 1.0
    ones1 = np.ones((1, 128), dtype=np.float32)
    padrow = np.zeros((1, CW), dtype=np.float32)
    padrow[0, M:128] = NEG

    masks = np.zeros((128, NSLOT, 640), dtype=np.float32)
    for si, qb in enumerate(qbs):
        J = SLOT_J[si]
        base = J - 640
        for gg in range(16):
            for isub in range(8):
                i_glob = qb * 128 + gg * 8 + isub
                jmax = 128 + i_glob + 1
                cols = np.arange(base, J)
                masks[gg * 8 + isub, si, cols >= jmax] = NEG

    wo_r = np.zeros((128, 8, DIM), dtype=np.float32)
    for ko in range(H):
        wo_r[(ko % 2) * 64:(ko % 2) * 64 + 64, ko // 2, :] = \
            Wo[ko * 64:(ko + 1) * 64, :]

    f = np.ascontiguousarray
    return {
        "xT_hi": f(_chunk_part(xT_hi)), "xT_lo": f(_chunk_part(xT_lo)),
        "xqT_hi": f(_chunk_part(xqT_hi).reshape(128, 8, NSLOT, 128)),
        "xqT_lo": f(_chunk_part(xqT_lo).reshape(128, 8, NSLOT, 128)),
        "wq_hi": f(_chunk_part(wq_hi)), "wq_lo": f(_chunk_part(wq_lo)),
        "wk_hi": f(_chunk_part(wk_hi)), "wk_lo": f(_chunk_part(wk_lo)),
        "wv": f(_chunk_part(Wv.astype(bf16))),
        "wo": f(wo_r.astype(bf16)),
        "memKT_hi": f(mkt_hi), "memKT_lo": f(mkt_lo),
        "memV": f(mv.astype(bf16)),
        "w1": w1, "w2": f(w2.astype(bf16)), "selg": selg, "ones1": ones1,
        "padrow": padrow, "masks": masks,
        "bo_in": f(np.broadcast_to(bo[None, :], (128, DIM)).copy()),
    }


def kernel(**inputs) -> np.ndarray:
    from concourse.bass_utils import run_bass_kernel_spmd

    _prep_cache.clear()

    if "nc" not in _nc_cache:
        _nc_cache["nc"] = build_nc()
    nc = _nc_cache["nc"]

    in_maps = [_host_prep(c, inputs) for c in range(8)]
    res = run_bass_kernel_spmd(nc, in_maps, core_ids=list(range(8)))

    out = np.zeros((B, N, DIM), dtype=np.float32)
    for c in range(8):
        b = c // 4
        qbs = QBS_OF_G[c % 4]
        o = res.results[c]["out"]
        for si, qb in enumerate(qbs):
            out[b, qb * 128:(qb + 1) * 128, :] = o[si]
    return out



# revision 14
# speedup vs baseline: 3.7348x; 3.7348x over previous
"""Sparse attention (talking-heads + top-64) Trainium2 kernel, 8-core SPMD.

Sharding: (batch, query-block) across 8 cores. Core c handles batch c//4;
its group index g4=c%4 selects query-blocks [g4, 7-g4, 8+g4, 15-g4], one
per slot; slot s is compiled for the max J of its qb group so the SPMD
program is identical on every core.

v2 design: minimize instruction count (the terminal charges ~30-60us per
instruction regardless of size). All-bf16 single-term projections, dots
matrix resident in SBUF (no DRAM round trip), 512-wide matmul chunks with
batched multi-bank PSUM evacuations, flat bf16 top-64 per row-group
(8x max8 + 7x match_replace), fused scalar_tensor_tensor softmax with
accum_out denominator, dma_start_transpose for the p-matrix transpose,
causal masking via host-precomputed mask rows added post-mix.

Layouts (per core):
  qt   [128 c, 8 ic, 512 q]     bf16   Q^T, scale folded into Wq
  kt   [128 c, 8 ic, 2176 j]    bf16   K^T with mem keys at j<16, pad 16:128
  v_sb [128 j, 17 jp, 1024 kd]  bf16   V rows (chunk 0 = mem)
  A    [128, 16 g, 2176]        bf16   dm logits -> p~ -> p^T (in place)
  dots_c [128, 16 h, 1024]      bf16   QK chunk before head-shuffle
  row-group g rows: p = i_sub*16 + head  (i_sub = p//16)
"""
import numpy as np
import ml_dtypes

bf16 = ml_dtypes.bfloat16

B, N, DIM = 2, 2048, 1024
H, D, M = 16, 64, 16
TOPK = 64
NEG = -1.0e30
SLOT_J = [640, 1152, 1664, 2176]
NSLOT = 4
QBS_OF_G = [[g, 7 - g, 8 + g, 15 - g] for g in range(4)]

_nc_cache = {}
_prep_cache = {}


def build_nc(reps=1):
    import concourse.tile as tile
    import concourse.mybir as mybir
    from concourse import bacc

    dt = mybir.dt
    nc = bacc.Bacc()
    AF = mybir.ActivationFunctionType
    ALU = mybir.AluOpType

    def din(name, shape, d=dt.bfloat16):
        return nc.dram_tensor(name, shape, d, kind="ExternalInput")

    xT_in = din("xT", [128, 8, N])
    xqT_in = din("xqT", [128, 8, 512])
    wq_in = din("wq", [128, 8, DIM])
    wk_in = din("wk", [128, 8, DIM])
    wv_in = din("wv", [128, 8, DIM])
    wo_in = din("wo", [128, 8, DIM])
    memKT_in = din("memKT", [128, 8, 128])
    memV_in = din("memV", [128, DIM])
    w1_in = din("w1", [128, 128])
    w2_in = din("w2", [128, 128])
    masks_in = din("masks", [128, NSLOT, 640])
    bo_in = din("bo", [128, DIM], dt.float32)

    out_d = nc.dram_tensor("out", [NSLOT, 128, DIM], dt.float32,
                           kind="ExternalOutput")

    with tile.TileContext(nc) as tc:
      for _rep in range(reps):
        with tc.tile_pool(name="persist", bufs=1) as pw:
            kt = pw.tile([128, 8, 2176], dt.bfloat16)
            v_sb = pw.tile([128, 17, DIM], dt.bfloat16)
            qt = pw.tile([128, 8, 512], dt.bfloat16)
            wo_s = pw.tile([128, 8, DIM], dt.bfloat16)
            w1_s = pw.tile([128, 128], dt.bfloat16)
            w2_s = pw.tile([128, 128], dt.bfloat16)
            masks_s = pw.tile([128, NSLOT, 640], dt.bfloat16)
            bo_s = pw.tile([128, DIM], dt.float32)

            nc.sync.dma_start(wo_s[:], wo_in[:])
            nc.sync.dma_start(w1_s[:], w1_in[:])
            nc.sync.dma_start(w2_s[:], w2_in[:])
            nc.sync.dma_start(masks_s[:], masks_in[:])
            nc.sync.dma_start(bo_s[:], bo_in[:])
            nc.sync.dma_start(kt[:, :, 0:128], memKT_in[:])
            nc.sync.dma_start(v_sb[:, 0, :], memV_in[:])

            # ---------------- phase 1: QT, KT, V projections -------------
            with tc.tile_pool(name="ph1", bufs=1) as p1, \
                 tc.tile_pool(name="ps1", bufs=1, space="PSUM") as ps1:
                xT = p1.tile([128, 8, N], dt.bfloat16)
                xqT = p1.tile([128, 8, 512], dt.bfloat16)
                wq_s = p1.tile([128, 8, DIM], dt.bfloat16)
                wk_s = p1.tile([128, 8, DIM], dt.bfloat16)
                wv_s = p1.tile([128, 8, DIM], dt.bfloat16)
                nc.sync.dma_start(xT[:], xT_in[:])
                nc.sync.dma_start(xqT[:], xqT_in[:])
                nc.sync.dma_start(wq_s[:], wq_in[:])
                nc.sync.dma_start(wk_s[:], wk_in[:])
                nc.sync.dma_start(wv_s[:], wv_in[:])

                # QT: [128 ic, 512 q] per ic
                for ic in range(8):
                    psq = ps1.tile([128, 512], dt.float32, tag="psq")
                    for dc in range(8):
                        nc.tensor.matmul(
                            psq[:], wq_s[:, dc, ic * 128:(ic + 1) * 128],
                            xqT[:, dc, :], start=(dc == 0), stop=(dc == 7))
                    nc.scalar.copy(qt[:, ic, :], psq[:])

                # KT: [128 ic, 2048 j]
                for ic in range(8):
                    psk = ps1.tile([128, 4, 512], dt.float32, tag="psk")
                    for jc in range(4):
                        for dc in range(8):
                            nc.tensor.matmul(
                                psk[:, jc, :],
                                wk_s[:, dc, ic * 128:(ic + 1) * 128],
                                xT[:, dc, jc * 512:(jc + 1) * 512],
                                start=(dc == 0), stop=(dc == 7))
                    nc.scalar.copy(kt[:, ic, 128:2176], psk[:])

                # V: [128 j, 1024 kd] per 128-row chunk
                for jp in range(16):
                    psv = ps1.tile([128, 2, 512], dt.float32, tag="psv")
                    for dc in range(8):
                        for fh in range(2):
                            nc.tensor.matmul(
                                psv[:, fh, :],
                                xT[:, dc, jp * 128:(jp + 1) * 128],
                                wv_s[:, dc, fh * 512:(fh + 1) * 512],
                                start=(dc == 0), stop=(dc == 7))
                    nc.scalar.copy(v_sb[:, 1 + jp, :], psv[:])

            # ---------------- per-slot: QK + mix + attn + AV + Wo --------
            with tc.tile_pool(name="pmain", bufs=1) as pm:
              A = pm.tile([128, 16, 2176], dt.bfloat16)
              for s in range(NSLOT):
                J = SLOT_J[s]
                NJP = J // 128
                NC5 = (J + 511) // 512   # 512-wide chunks
                NC2 = (J + 767) // 768   # 768-wide dots chunks

                # ---- QK + talking-heads pre-mix -> A (dm) ----
                with tc.tile_pool(name="qkm", bufs=1) as pq, \
                     tc.tile_pool(name="shufp", bufs=1) as pshuf, \
                     tc.tile_pool(name="psQK", bufs=1, space="PSUM") as psqk, \
                     tc.tile_pool(name="psMix", bufs=1, space="PSUM") as psmx:
                    for c2 in range(NC2):
                        j0 = c2 * 768
                        W2 = min(768, J - j0)
                        ncs = (W2 + 511) // 512
                        dots_c = pq.tile([128, 16, 768], dt.bfloat16,
                                         tag="dots")
                        for cs in range(ncs):
                            W5 = min(512, W2 - cs * 512)
                            for h4 in range(4):
                                psd = psqk.tile([128, 4, 512], dt.float32,
                                                tag="psd")
                                for hh in range(4):
                                    h = h4 * 4 + hh
                                    hp = (h % 2) * 64
                                    nc.tensor.matmul(
                                        psd[:, hh, 0:W5],
                                        qt[hp:hp + 64, h // 2,
                                           s * 128:(s + 1) * 128],
                                        kt[hp:hp + 64, h // 2,
                                           j0 + cs * 512:j0 + cs * 512 + W5],
                                        start=True, stop=True)
                                nc.scalar.copy(
                                    dots_c[:, h4 * 4:h4 * 4 + 4,
                                           cs * 512:cs * 512 + W5],
                                    psd[:, :, 0:W5])
                        for gq in range(4):
                            shufs = []
                            for gi in range(4):
                                g = gq * 4 + gi
                                shuf = pshuf.tile([128, 768], dt.bfloat16,
                                                  tag=f"shuf{gi}",
                                                  name=f"shuf{gi}")
                                nc.gpsimd.dma_start(
                                    shuf[:, 0:W2],
                                    dots_c[g * 8:(g + 1) * 8, :, 0:W2])
                                shufs.append(shuf)
                            for cs in range(ncs):
                                W5 = min(512, W2 - cs * 512)
                                psm = psmx.tile([128, 4, 512], dt.float32,
                                                tag="psm")
                                for gi in range(4):
                                    nc.tensor.matmul(
                                        psm[:, gi, 0:W5], w1_s[:],
                                        shufs[gi][:, cs * 512:cs * 512 + W5],
                                        start=True, stop=True)
                                nc.scalar.copy(
                                    A[:, gq * 4:gq * 4 + 4,
                                      j0 + cs * 512:j0 + cs * 512 + W5],
                                    psm[:, :, 0:W5])

                # ---- causal masks + pad mask ----
                nc.vector.memset(A[:, :, 16:128], NEG)
                for g in range(16):
                    woff = J - 640 + 8 * g
                    nc.gpsimd.tensor_add(
                        A[:, g, woff:J], A[:, g, woff:J],
                        masks_s[:, s, 0:640 - 8 * g])

                # ---- per row-group: top-64 + softmax -> p~ (in place) ----
                with tc.tile_pool(name="att", bufs=1) as pa, \
                     tc.tile_pool(name="psP", bufs=1, space="PSUM") as psp:
                    for g in range(16):
                        dm = A[:, g, 0:J]
                        scr = pa.tile([128, 2176], dt.bfloat16, tag="scr")
                        cands = pa.tile([128, 64], dt.float32, tag="cands")
                        nc.vector.max(cands[:, 0:8], dm)
                        nc.vector.match_replace(scr[:, 0:J], cands[:, 0:8],
                                                dm, NEG)
                        for r in range(1, 8):
                            nc.vector.max(cands[:, r * 8:(r + 1) * 8],
                                          scr[:, 0:J])
                            if r < 7:
                                nc.vector.match_replace(
                                    scr[:, 0:J], cands[:, r * 8:(r + 1) * 8],
                                    scr[:, 0:J], NEG)
                        negm = pa.tile([128, 1], dt.float32, tag="negm")
                        nc.vector.tensor_scalar_mul(negm[:], cands[:, 0:1],
                                                    -1.0)
                        p2 = pa.tile([128, 2176], dt.bfloat16, tag="p2")
                        nc.scalar.activation(p2[:, 0:J], dm, AF.Exp,
                                             bias=negm[:])
                        den = pa.tile([128, 1], dt.float32, tag="den")
                        nc.vector.scalar_tensor_tensor(
                            dm, dm, cands[:, 63:64], p2[:, 0:J],
                            op0=ALU.is_ge, op1=ALU.mult, accum_out=den[:])
                        rden = pa.tile([128, 1], dt.float32, tag="rden")
                        nc.vector.reciprocal(rden[:], den[:])
                        nc.vector.tensor_scalar_mul(dm, dm, rden[:])

                        # post-mix (w2) -> B2, then transpose back into A
                        b2 = pa.tile([128, 2176], dt.bfloat16, tag="b2")
                        nps = (NC5 + 3) // 4
                        for q4 in range(nps):
                            pspost = psp.tile([128, 4, 512], dt.float32,
                                              tag="pspost")
                            nq = min(4, NC5 - q4 * 4)
                            for qq in range(nq):
                                c5 = q4 * 4 + qq
                                W5 = min(512, J - c5 * 512)
                                nc.tensor.matmul(
                                    pspost[:, qq, 0:W5], w2_s[:],
                                    A[:, g, c5 * 512:c5 * 512 + W5],
                                    start=True, stop=True)
                            W4 = min(2048, J - q4 * 2048)
                            nc.vector.tensor_copy(
                                b2[:, q4 * 2048:q4 * 2048 + W4],
                                pspost.rearrange("p a b -> p (a b)")[:, 0:W4])
                        nc.sync.dma_start_transpose(
                            A[:, g, 0:J].rearrange("p (a b) -> p a b", b=128),
                            b2[:, 0:J])

                    # ---- AV ----
                    psav = psp.tile([128, 8, 128], dt.float32, tag="psav")
                    for k in range(16):
                        kp = (k % 2) * 64
                        for jp in range(NJP):
                            nc.tensor.matmul(
                                psav[kp:kp + 64, k // 2, :],
                                v_sb[:, jp, k * 64:(k + 1) * 64],
                                A[:, :, jp * 128 + k:jp * 128 + k + 113:16],
                                start=(jp == 0), stop=(jp == NJP - 1))
                    av = pa.tile([128, 8, 128], dt.bfloat16, tag="av")
                    nc.vector.tensor_copy(av[:], psav[:])

                    # ---- Wo projection + bias ----
                    pso = psp.tile([128, 2, 512], dt.float32, tag="pso")
                    for kk in range(8):
                        for fh in range(2):
                            nc.tensor.matmul(
                                pso[:, fh, :], av[:, kk, :],
                                wo_s[:, kk, fh * 512:(fh + 1) * 512],
                                start=(kk == 0), stop=(kk == 7))
                    osb = pa.tile([128, DIM], dt.float32, tag="osb")
                    nc.vector.tensor_add(
                        osb[:], pso.rearrange("p a b -> p (a b)")[:], bo_s[:])
                    nc.sync.dma_start(out_d[s, :, :], osb[:])

    nc.finalize()
    return nc


def _host_prep(core, inputs):
    x = np.asarray(inputs["x"], dtype=np.float32)
    Wq = np.asarray(inputs["Wq"], dtype=np.float32) * (D ** -0.5)
    Wk = np.asarray(inputs["Wk"], dtype=np.float32)
    Wv = np.asarray(inputs["Wv"], dtype=np.float32)
    Wo = np.asarray(inputs["Wo"], dtype=np.float32)
    bo = np.asarray(inputs["bo"], dtype=np.float32)
    pre = np.asarray(inputs["pre_proj"], dtype=np.float32)
    post = np.asarray(inputs["post_proj"], dtype=np.float32)
    mem_k = np.asarray(inputs["mem_k"], dtype=np.float32)
    mem_v = np.asarray(inputs["mem_v"], dtype=np.float32)

    b = core // 4
    g4 = core % 4
    qbs = QBS_OF_G[g4]

    xb = x[b]
    if ("xT", b) not in _prep_cache:
        xT = np.ascontiguousarray(xb.T).astype(bf16)  # [DIM, N]
        _prep_cache[("xT", b)] = np.ascontiguousarray(
            xT.reshape(8, 128, N).transpose(1, 0, 2))
    xT_c = _prep_cache[("xT", b)]

    xq = np.concatenate([xb[qb * 128:(qb + 1) * 128] for qb in qbs], axis=0)
    xqT = np.ascontiguousarray(xq.T).astype(bf16)     # [DIM, 512]
    xqT_c = np.ascontiguousarray(xqT.reshape(8, 128, 512).transpose(1, 0, 2))

    def chunk_w(w):
        return np.ascontiguousarray(
            w.astype(bf16).reshape(8, 128, DIM).transpose(1, 0, 2))

    if "w" not in _prep_cache:
        wo_r = np.zeros((128, 8, DIM), dtype=np.float32)
        for k in range(H):
            wo_r[(k % 2) * 64:(k % 2) * 64 + 64, k // 2, :] = \
                Wo[k * 64:(k + 1) * 64, :]

        mkt = np.zeros((128, 8, 128), dtype=np.float32)
        for h in range(H):
            # kt[p, ic, j] = K[j, ic*128+p]; mem key j<16, inner c = h*64+d
            for dd in range(D):
                c = h * 64 + dd
                mkt[c % 128, c // 128, 0:M] = mem_k[h, :, dd]
        mv = np.zeros((128, DIM), dtype=np.float32)
        mv[0:M] = mem_v.transpose(1, 0, 2).reshape(M, DIM)

        w1 = np.zeros((128, 128), dtype=np.float32)
        w2 = np.zeros((128, 128), dtype=np.float32)
        for i in range(8):
            for h in range(H):
                for k in range(H):
                    w1[i * 16 + h, i * 16 + k] = pre[h, k]
                    w2[i * 16 + h, i * 16 + k] = post[h, k]

        _prep_cache["w"] = (
            chunk_w(Wq), chunk_w(Wk), chunk_w(Wv),
            wo_r.astype(bf16), mkt.astype(bf16), mv.astype(bf16),
            w1.astype(bf16), w2.astype(bf16),
            np.ascontiguousarray(
                np.broadcast_to(bo[None, :], (128, DIM)).copy()))
    (wq_c, wk_c, wv_c, wo_r, mkt, mv, w1, w2, bo_b) = _prep_cache["w"]

    masks = np.zeros((128, NSLOT, 640), dtype=np.float32)
    for si, qb in enumerate(qbs):
        qbmax = max(QBS_OF_G[gg][si] for gg in range(4))
        brel_base = 513 - 128 * (qbmax - qb)
        cols = np.arange(640)
        for p in range(128):
            i = p // 16
            masks[p, si, cols >= brel_base + i] = NEG

    return {
        "xT": xT_c, "xqT": xqT_c,
        "wq": wq_c, "wk": wk_c, "wv": wv_c, "wo": wo_r,
        "memKT": mkt, "memV": mv,
        "w1": w1, "w2": w2,
        "masks": masks.astype(bf16),
        "bo": bo_b,
    }


def kernel(**inputs) -> np.ndarray:
    from concourse.bass_utils import run_bass_kernel_spmd

    _prep_cache.clear()

    if "nc" not in _nc_cache:
        _nc_cache["nc"] = build_nc()
    nc = _nc_cache["nc"]

    in_maps = [_host_prep(c, inputs) for c in range(8)]
    res = run_bass_kernel_spmd(nc, in_maps, core_ids=list(range(8)))

    out = np.zeros((B, N, DIM), dtype=np.float32)
    for c in range(8):
        b = c // 4
        qbs = QBS_OF_G[c % 4]
        o = res.results[c]["out"]
        for si, qb in enumerate(qbs):
            out[b, qb * 128:(qb + 1) * 128, :] = o[si]
    return out


# revision 16
# speedup vs baseline: 4.1321x; 1.1064x over previous
"""Sparse attention (talking-heads + top-64) Trainium2 kernel, 8-core SPMD.

Sharding: (batch, query-block) across 8 cores. Core c handles batch c//4;
its group index g4=c%4 selects query-blocks [g4, 7-g4, 8+g4, 15-g4], one
per slot; slot s is compiled for the max J of its qb group so the SPMD
program is identical on every core.

v2 design: minimize instruction count (the terminal charges ~30-60us per
instruction regardless of size). All-bf16 single-term projections, dots
matrix resident in SBUF (no DRAM round trip), 512-wide matmul chunks with
batched multi-bank PSUM evacuations, flat bf16 top-64 per row-group
(8x max8 + 7x match_replace), fused scalar_tensor_tensor softmax with
accum_out denominator, dma_start_transpose for the p-matrix transpose,
causal masking via host-precomputed mask rows added post-mix.

Layouts (per core):
  qt   [128 c, 8 ic, 512 q]     bf16   Q^T, scale folded into Wq
  kt   [128 c, 8 ic, 2176 j]    bf16   K^T with mem keys at j<16, pad 16:128
  v_sb [128 j, 17 jp, 1024 kd]  bf16   V rows (chunk 0 = mem)
  A    [128, 16 g, 2176]        bf16   dm logits -> p~ -> p^T (in place)
  dots_c [128, 16 h, 1024]      bf16   QK chunk before head-shuffle
  row-group g rows: p = i_sub*16 + head  (i_sub = p//16)
"""
import numpy as np
import ml_dtypes

bf16 = ml_dtypes.bfloat16

B, N, DIM = 2, 2048, 1024
H, D, M = 16, 64, 16
TOPK = 64
NEG = -1.0e30
SLOT_J = [640, 1152, 1664, 2176]
NSLOT = 4
QBS_OF_G = [[g, 7 - g, 8 + g, 15 - g] for g in range(4)]

_nc_cache = {}
_prep_cache = {}


def build_nc(reps=1, ablate=()):
    ab = set(ablate)
    import concourse.tile as tile
    import concourse.mybir as mybir
    from concourse import bacc

    dt = mybir.dt
    nc = bacc.Bacc()
    AF = mybir.ActivationFunctionType
    ALU = mybir.AluOpType

    def din(name, shape, d=dt.bfloat16):
        return nc.dram_tensor(name, shape, d, kind="ExternalInput")

    xT_in = din("xT", [128, 8, N])
    xqT_in = din("xqT", [128, 8, 512])
    wq_in = din("wq", [128, 8, DIM])
    wk_in = din("wk", [128, 8, DIM])
    wv_in = din("wv", [128, 8, DIM])
    wo_in = din("wo", [128, 8, DIM])
    memKT_in = din("memKT", [128, 8, 128])
    memV_in = din("memV", [128, DIM])
    w1_in = din("w1", [128, 128])
    w2_in = din("w2", [128, 128])
    masks_in = din("masks", [128, NSLOT, 640])
    bo_in = din("bo", [128, DIM], dt.float32)

    out_d = nc.dram_tensor("out", [NSLOT, 128, DIM], dt.float32,
                           kind="ExternalOutput")

    with tile.TileContext(nc) as tc:
      for _rep in range(reps):
        with tc.tile_pool(name="persist", bufs=1) as pw:
            kt = pw.tile([128, 8, 2176], dt.bfloat16)
            v_sb = pw.tile([128, 17, DIM], dt.bfloat16)
            qt = pw.tile([128, 8, 512], dt.bfloat16)
            wo_s = pw.tile([128, 8, DIM], dt.bfloat16)
            w1_s = pw.tile([128, 128], dt.bfloat16)
            w2_s = pw.tile([128, 128], dt.bfloat16)
            masks_s = pw.tile([128, NSLOT, 640], dt.bfloat16)
            bo_s = pw.tile([128, DIM], dt.float32)

            nc.sync.dma_start(wo_s[:], wo_in[:])
            nc.sync.dma_start(w1_s[:], w1_in[:])
            nc.sync.dma_start(w2_s[:], w2_in[:])
            nc.sync.dma_start(masks_s[:], masks_in[:])
            nc.sync.dma_start(bo_s[:], bo_in[:])
            nc.sync.dma_start(kt[:, :, 0:128], memKT_in[:])
            nc.sync.dma_start(v_sb[:, 0, :], memV_in[:])

            # ---------------- phase 1: QT, KT, V projections -------------
            with tc.tile_pool(name="ph1", bufs=1) as p1, \
                 tc.tile_pool(name="ps1", bufs=1, space="PSUM") as ps1:
                xT = p1.tile([128, 8, N], dt.bfloat16)
                xqT = p1.tile([128, 8, 512], dt.bfloat16)
                wq_s = p1.tile([128, 8, DIM], dt.bfloat16)
                wk_s = p1.tile([128, 8, DIM], dt.bfloat16)
                wv_s = p1.tile([128, 8, DIM], dt.bfloat16)
                nc.sync.dma_start(xT[:], xT_in[:])
                nc.sync.dma_start(xqT[:], xqT_in[:])
                nc.sync.dma_start(wq_s[:], wq_in[:])
                nc.sync.dma_start(wk_s[:], wk_in[:])
                nc.sync.dma_start(wv_s[:], wv_in[:])

                if "phase1" in ab:
                    nc.vector.memset(qt[:], 0.01)
                    nc.vector.memset(kt[:, :, 128:2176], 0.01)
                    nc.vector.memset(v_sb[:, 1:17, :], 0.01)
                # QT: [128 ic, 512 q] per ic
                for ic in range(8 if "phase1" not in ab else 0):
                    psq = ps1.tile([128, 512], dt.float32, tag="psq")
                    for dc in range(8):
                        nc.tensor.matmul(
                            psq[:], wq_s[:, dc, ic * 128:(ic + 1) * 128],
                            xqT[:, dc, :], start=(dc == 0), stop=(dc == 7))
                    nc.scalar.copy(qt[:, ic, :], psq[:])

                # KT: [128 ic, 2048 j]
                for ic in range(8 if "phase1" not in ab else 0):
                    psk = ps1.tile([128, 4, 512], dt.float32, tag="psk")
                    for jc in range(4):
                        for dc in range(8):
                            nc.tensor.matmul(
                                psk[:, jc, :],
                                wk_s[:, dc, ic * 128:(ic + 1) * 128],
                                xT[:, dc, jc * 512:(jc + 1) * 512],
                                start=(dc == 0), stop=(dc == 7))
                    nc.scalar.copy(kt[:, ic, 128:2176], psk[:])

                # V: [128 j, 1024 kd] per 128-row chunk
                for jp in range(16 if "phase1" not in ab else 0):
                    psv = ps1.tile([128, 2, 512], dt.float32, tag="psv")
                    for dc in range(8):
                        for fh in range(2):
                            nc.tensor.matmul(
                                psv[:, fh, :],
                                xT[:, dc, jp * 128:(jp + 1) * 128],
                                wv_s[:, dc, fh * 512:(fh + 1) * 512],
                                start=(dc == 0), stop=(dc == 7))
                    nc.scalar.copy(v_sb[:, 1 + jp, :], psv[:])

            # ---------------- per-slot: QK + mix + attn + AV + Wo --------
            with tc.tile_pool(name="pmain", bufs=1) as pm:
              A = pm.tile([128, 16, 2176], dt.bfloat16)
              for s in range(NSLOT):
                J = SLOT_J[s]
                NJP = J // 128
                NC5 = (J + 511) // 512   # 512-wide chunks
                NC2 = (J + 767) // 768   # 768-wide dots chunks

                # ---- QK + talking-heads pre-mix -> A (dm) ----
                with tc.tile_pool(name="qkm", bufs=1) as pq, \
                     tc.tile_pool(name="shufp", bufs=1) as pshuf, \
                     tc.tile_pool(name="psQK", bufs=1, space="PSUM") as psqk, \
                     tc.tile_pool(name="psMix", bufs=1, space="PSUM") as psmx:
                    for c2 in range(NC2):
                        j0 = c2 * 768
                        W2 = min(768, J - j0)
                        ncs = (W2 + 511) // 512
                        dots_c = pq.tile([128, 16, 768], dt.bfloat16,
                                         tag="dots")
                        if "qk" in ab:
                            nc.vector.memset(dots_c[:], 0.01)
                        for cs in range(ncs if "qk" not in ab else 0):
                            W5 = min(512, W2 - cs * 512)
                            for h4 in range(4):
                                psd = psqk.tile([128, 4, 512], dt.float32,
                                                tag="psd")
                                for hh in range(4):
                                    h = h4 * 4 + hh
                                    hp = (h % 2) * 64
                                    nc.tensor.matmul(
                                        psd[:, hh, 0:W5],
                                        qt[hp:hp + 64, h // 2,
                                           s * 128:(s + 1) * 128],
                                        kt[hp:hp + 64, h // 2,
                                           j0 + cs * 512:j0 + cs * 512 + W5],
                                        start=True, stop=True)
                                nc.scalar.copy(
                                    dots_c[:, h4 * 4:h4 * 4 + 4,
                                           cs * 512:cs * 512 + W5],
                                    psd[:, :, 0:W5])
                        if "mix" in ab:
                            nc.scalar.copy(A[:, :, j0:j0 + W2],
                                           dots_c[:, :, 0:W2])
                        for gq in range(4 if "mix" not in ab else 0):
                            shufs = []
                            for gi in range(4):
                                g = gq * 4 + gi
                                shuf = pshuf.tile([128, 768], dt.bfloat16,
                                                  tag=f"shuf{gi}",
                                                  name=f"shuf{gi}")
                                nc.gpsimd.dma_start(
                                    shuf[:, 0:W2],
                                    dots_c[g * 8:(g + 1) * 8, :, 0:W2])
                                shufs.append(shuf)
                            for cs in range(ncs):
                                W5 = min(512, W2 - cs * 512)
                                psm = psmx.tile([128, 4, 512], dt.float32,
                                                tag="psm")
                                for gi in range(4):
                                    nc.tensor.matmul(
                                        psm[:, gi, 0:W5], w1_s[:],
                                        shufs[gi][:, cs * 512:cs * 512 + W5],
                                        start=True, stop=True)
                                nc.scalar.copy(
                                    A[:, gq * 4:gq * 4 + 4,
                                      j0 + cs * 512:j0 + cs * 512 + W5],
                                    psm[:, :, 0:W5])

                # ---- causal masks + pad mask ----
                nc.vector.memset(A[:, :, 16:128], NEG)
                for g in range(16):
                    woff = J - 640 + 8 * g
                    nc.gpsimd.tensor_add(
                        A[:, g, woff:J], A[:, g, woff:J],
                        masks_s[:, s, 0:640 - 8 * g])

                # ---- per row-group: top-64 + softmax -> p~ (in place) ----
                with tc.tile_pool(name="att", bufs=1) as pa, \
                     tc.tile_pool(name="psP", bufs=1, space="PSUM") as psp:
                    for g in range(16):
                        dm = A[:, g, 0:J]
                        scr = pa.tile([128, 2176], dt.bfloat16, tag="scr")
                        cands = pa.tile([128, 64], dt.float32, tag="cands")
                        if "topk" in ab:
                            nc.vector.memset(cands[:], 0.0)
                        else:
                            nc.vector.max(cands[:, 0:8], dm)
                            nc.vector.match_replace(scr[:, 0:J],
                                                    cands[:, 0:8], dm, NEG)
                            for r in range(1, 8):
                                nc.vector.max(cands[:, r * 8:(r + 1) * 8],
                                              scr[:, 0:J])
                                if r < 7:
                                    nc.vector.match_replace(
                                        scr[:, 0:J],
                                        cands[:, r * 8:(r + 1) * 8],
                                        scr[:, 0:J], NEG)
                        negm = pa.tile([128, 1], dt.float32, tag="negm")
                        nc.vector.tensor_scalar_mul(negm[:], cands[:, 0:1],
                                                    -1.0)
                        p2 = pa.tile([128, 2176], dt.bfloat16, tag="p2")
                        nc.scalar.activation(p2[:, 0:J], dm, AF.Exp,
                                             bias=negm[:])
                        den = pa.tile([128, 1], dt.float32, tag="den")
                        nc.vector.scalar_tensor_tensor(
                            dm, dm, cands[:, 63:64], p2[:, 0:J],
                            op0=ALU.is_ge, op1=ALU.mult, accum_out=den[:])
                        rden = pa.tile([128, 1], dt.float32, tag="rden")
                        nc.vector.reciprocal(rden[:], den[:])
                        nc.vector.tensor_scalar_mul(dm, dm, rden[:])

                        # post-mix (w2) -> B2, then transpose back into A
                        b2 = pa.tile([128, 2176], dt.bfloat16, tag="b2")
                        if "postmix" in ab:
                            nc.vector.memset(b2[:, 0:J], 0.01)
                        nps = (NC5 + 3) // 4
                        for q4 in range(nps if "postmix" not in ab else 0):
                            pspost = psp.tile([128, 4, 512], dt.float32,
                                              tag="pspost")
                            nq = min(4, NC5 - q4 * 4)
                            for qq in range(nq):
                                c5 = q4 * 4 + qq
                                W5 = min(512, J - c5 * 512)
                                nc.tensor.matmul(
                                    pspost[:, qq, 0:W5], w2_s[:],
                                    A[:, g, c5 * 512:c5 * 512 + W5],
                                    start=True, stop=True)
                            W4 = min(2048, J - q4 * 2048)
                            nc.vector.tensor_copy(
                                b2[:, q4 * 2048:q4 * 2048 + W4],
                                pspost.rearrange("p a b -> p (a b)")[:, 0:W4])
                        if "transpose" not in ab:
                            nc.sync.dma_start_transpose(
                                A[:, g, 0:J].rearrange("p (a b) -> p a b",
                                                       b=128),
                                b2[:, 0:J])

                    # ---- AV ----
                    psav = psp.tile([128, 8, 128], dt.float32, tag="psav")
                    if "av" in ab:
                        nc.vector.memset(psav[:], 0.01)
                    for k in range(16 if "av" not in ab else 0):
                        kp = (k % 2) * 64
                        for jp in range(NJP):
                            nc.tensor.matmul(
                                psav[kp:kp + 64, k // 2, :],
                                v_sb[:, jp, k * 64:(k + 1) * 64],
                                A[:, :, jp * 128 + k:jp * 128 + k + 113:16],
                                start=(jp == 0), stop=(jp == NJP - 1))
                    av = pa.tile([128, 8, 128], dt.bfloat16, tag="av")
                    nc.vector.tensor_copy(av[:], psav[:])

                    # ---- Wo projection + bias ----
                    pso = psp.tile([128, 2, 512], dt.float32, tag="pso")
                    for kk in range(8):
                        for fh in range(2):
                            nc.tensor.matmul(
                                pso[:, fh, :], av[:, kk, :],
                                wo_s[:, kk, fh * 512:(fh + 1) * 512],
                                start=(kk == 0), stop=(kk == 7))
                    osb = pa.tile([128, DIM], dt.float32, tag="osb")
                    nc.vector.tensor_add(
                        osb[:], pso.rearrange("p a b -> p (a b)")[:], bo_s[:])
                    nc.sync.dma_start(out_d[s, :, :], osb[:])

    nc.finalize()
    return nc


def _host_prep(core, inputs):
    x = np.asarray(inputs["x"], dtype=np.float32)
    Wq = np.asarray(inputs["Wq"], dtype=np.float32) * (D ** -0.5)
    Wk = np.asarray(inputs["Wk"], dtype=np.float32)
    Wv = np.asarray(inputs["Wv"], dtype=np.float32)
    Wo = np.asarray(inputs["Wo"], dtype=np.float32)
    bo = np.asarray(inputs["bo"], dtype=np.float32)
    pre = np.asarray(inputs["pre_proj"], dtype=np.float32)
    post = np.asarray(inputs["post_proj"], dtype=np.float32)
    mem_k = np.asarray(inputs["mem_k"], dtype=np.float32)
    mem_v = np.asarray(inputs["mem_v"], dtype=np.float32)

    b = core // 4
    g4 = core % 4
    qbs = QBS_OF_G[g4]

    xb = x[b]
    if ("xT", b) not in _prep_cache:
        xT = np.ascontiguousarray(xb.T).astype(bf16)  # [DIM, N]
        _prep_cache[("xT", b)] = np.ascontiguousarray(
            xT.reshape(8, 128, N).transpose(1, 0, 2))
    xT_c = _prep_cache[("xT", b)]

    xq = np.concatenate([xb[qb * 128:(qb + 1) * 128] for qb in qbs], axis=0)
    xqT = np.ascontiguousarray(xq.T).astype(bf16)     # [DIM, 512]
    xqT_c = np.ascontiguousarray(xqT.reshape(8, 128, 512).transpose(1, 0, 2))

    def chunk_w(w):
        return np.ascontiguousarray(
            w.astype(bf16).reshape(8, 128, DIM).transpose(1, 0, 2))

    if "w" not in _prep_cache:
        wo_r = np.zeros((128, 8, DIM), dtype=np.float32)
        for k in range(H):
            wo_r[(k % 2) * 64:(k % 2) * 64 + 64, k // 2, :] = \
                Wo[k * 64:(k + 1) * 64, :]

        mkt = np.zeros((128, 8, 128), dtype=np.float32)
        for h in range(H):
            # kt[p, ic, j] = K[j, ic*128+p]; mem key j<16, inner c = h*64+d
            for dd in range(D):
                c = h * 64 + dd
                mkt[c % 128, c // 128, 0:M] = mem_k[h, :, dd]
        mv = np.zeros((128, DIM), dtype=np.float32)
        mv[0:M] = mem_v.transpose(1, 0, 2).reshape(M, DIM)

        w1 = np.zeros((128, 128), dtype=np.float32)
        w2 = np.zeros((128, 128), dtype=np.float32)
        for i in range(8):
            for h in range(H):
                for k in range(H):
                    w1[i * 16 + h, i * 16 + k] = pre[h, k]
                    w2[i * 16 + h, i * 16 + k] = post[h, k]

        _prep_cache["w"] = (
            chunk_w(Wq), chunk_w(Wk), chunk_w(Wv),
            wo_r.astype(bf16), mkt.astype(bf16), mv.astype(bf16),
            w1.astype(bf16), w2.astype(bf16),
            np.ascontiguousarray(
                np.broadcast_to(bo[None, :], (128, DIM)).copy()))
    (wq_c, wk_c, wv_c, wo_r, mkt, mv, w1, w2, bo_b) = _prep_cache["w"]

    masks = np.zeros((128, NSLOT, 640), dtype=np.float32)
    for si, qb in enumerate(qbs):
        qbmax = max(QBS_OF_G[gg][si] for gg in range(4))
        brel_base = 513 - 128 * (qbmax - qb)
        cols = np.arange(640)
        for p in range(128):
            i = p // 16
            masks[p, si, cols >= brel_base + i] = NEG

    return {
        "xT": xT_c, "xqT": xqT_c,
        "wq": wq_c, "wk": wk_c, "wv": wv_c, "wo": wo_r,
        "memKT": mkt, "memV": mv,
        "w1": w1, "w2": w2,
        "masks": masks.astype(bf16),
        "bo": bo_b,
    }


def kernel(**inputs) -> np.ndarray:
    from concourse.bass_utils import run_bass_kernel_spmd

    _prep_cache.clear()

    if "nc" not in _nc_cache:
        _nc_cache["nc"] = build_nc()
    nc = _nc_cache["nc"]

    in_maps = [_host_prep(c, inputs) for c in range(8)]
    res = run_bass_kernel_spmd(nc, in_maps, core_ids=list(range(8)))

    out = np.zeros((B, N, DIM), dtype=np.float32)
    for c in range(8):
        b = c // 4
        qbs = QBS_OF_G[c % 4]
        o = res.results[c]["out"]
        for si, qb in enumerate(qbs):
            out[b, qb * 128:(qb + 1) * 128, :] = o[si]
    return out


# revision 19
# speedup vs baseline: 4.2319x; 1.0242x over previous
"""Sparse attention (talking-heads + top-64) Trainium2 kernel, 8-core SPMD.

Sharding: (batch, query-block) across 8 cores. Core c handles batch c//4;
its group index g4=c%4 selects query-blocks [g4, 7-g4, 8+g4, 15-g4], one
per slot; slot s is compiled for the max J of its qb group so the SPMD
program is identical on every core.

v2 design: minimize instruction count (the terminal charges ~30-60us per
instruction regardless of size). All-bf16 single-term projections, dots
matrix resident in SBUF (no DRAM round trip), 512-wide matmul chunks with
batched multi-bank PSUM evacuations, flat bf16 top-64 per row-group
(8x max8 + 7x match_replace), fused scalar_tensor_tensor softmax with
accum_out denominator, dma_start_transpose for the p-matrix transpose,
causal masking via host-precomputed mask rows added post-mix.

Layouts (per core):
  qt   [128 c, 8 ic, 512 q]     bf16   Q^T, scale folded into Wq
  kt   [128 c, 8 ic, 2176 j]    bf16   K^T with mem keys at j<16, pad 16:128
  v_sb [128 j, 17 jp, 1024 kd]  bf16   V rows (chunk 0 = mem)
  A    [128, 16 g, 2176]        bf16   dm logits -> p~ -> p^T (in place)
  dots_c [128, 16 h, 1024]      bf16   QK chunk before head-shuffle
  row-group g rows: p = i_sub*16 + head  (i_sub = p//16)
"""
import numpy as np
import ml_dtypes

bf16 = ml_dtypes.bfloat16

B, N, DIM = 2, 2048, 1024
H, D, M = 16, 64, 16
TOPK = 64
NEG = -1.0e30
SLOT_J = [640, 1152, 1664, 2176]
NSLOT = 4
QBS_OF_G = [[g, 7 - g, 8 + g, 15 - g] for g in range(4)]

_nc_cache = {}
_prep_cache = {}


def build_nc(reps=1, ablate=()):
    ab = set(ablate)
    import concourse.tile as tile
    import concourse.mybir as mybir
    from concourse import bacc

    dt = mybir.dt
    nc = bacc.Bacc()
    AF = mybir.ActivationFunctionType
    ALU = mybir.AluOpType

    def din(name, shape, d=dt.bfloat16):
        return nc.dram_tensor(name, shape, d, kind="ExternalInput")

    xT_in = din("xT", [128, 8, N])
    xqT_in = din("xqT", [128, 8, 512])
    wq_in = din("wq", [128, 8, DIM])
    wk_in = din("wk", [128, 8, DIM])
    wv_in = din("wv", [128, 8, DIM])
    wo_in = din("wo", [128, 8, DIM])
    memKT_in = din("memKT", [128, 8, 128])
    memV_in = din("memV", [128, DIM])
    w1_in = din("w1", [128, 128])
    w2_in = din("w2", [128, 128])
    masks_in = din("masks", [128, NSLOT, 640])
    bo_in = din("bo", [128, DIM], dt.float32)

    out_d = nc.dram_tensor("out", [NSLOT, 128, DIM], dt.float32,
                           kind="ExternalOutput")

    with tile.TileContext(nc) as tc:
      for _rep in range(reps):
        with tc.tile_pool(name="persist", bufs=1) as pw:
            kt = pw.tile([128, 8, 2176], dt.bfloat16)
            v_sb = pw.tile([128, 17, DIM], dt.bfloat16)
            qt = pw.tile([128, 8, 512], dt.bfloat16)
            wo_s = pw.tile([128, 8, DIM], dt.bfloat16)
            w1_s = pw.tile([128, 128], dt.bfloat16)
            w2_s = pw.tile([128, 128], dt.bfloat16)
            masks_s = pw.tile([128, NSLOT, 640], dt.bfloat16)
            bo_s = pw.tile([128, DIM], dt.float32)

            nc.sync.dma_start(wo_s[:], wo_in[:])
            nc.sync.dma_start(w1_s[:], w1_in[:])
            nc.sync.dma_start(w2_s[:], w2_in[:])
            nc.sync.dma_start(masks_s[:], masks_in[:])
            nc.sync.dma_start(bo_s[:], bo_in[:])
            nc.sync.dma_start(kt[:, :, 0:128], memKT_in[:])
            nc.sync.dma_start(v_sb[:, 0, :], memV_in[:])

            # ---------------- phase 1: QT, KT, V projections -------------
            with tc.tile_pool(name="ph1", bufs=1) as p1, \
                 tc.tile_pool(name="ps1", bufs=1, space="PSUM") as ps1:
                xT = p1.tile([128, 8, N], dt.bfloat16)
                xqT = p1.tile([128, 8, 512], dt.bfloat16)
                wq_s = p1.tile([128, 8, DIM], dt.bfloat16)
                wk_s = p1.tile([128, 8, DIM], dt.bfloat16)
                wv_s = p1.tile([128, 8, DIM], dt.bfloat16)
                nc.sync.dma_start(xT[:], xT_in[:])
                nc.sync.dma_start(xqT[:], xqT_in[:])
                nc.sync.dma_start(wq_s[:], wq_in[:])
                nc.sync.dma_start(wk_s[:], wk_in[:])
                nc.sync.dma_start(wv_s[:], wv_in[:])

                if "phase1" in ab:
                    nc.vector.memset(qt[:], 0.01)
                    nc.vector.memset(kt[:, :, 128:2176], 0.01)
                    nc.vector.memset(v_sb[:, 1:17, :], 0.01)
                # QT: [128 ic, 512 q] per ic
                for ic in range(8 if "phase1" not in ab else 0):
                    psq = ps1.tile([128, 512], dt.float32, tag="psq")
                    for dc in range(8):
                        nc.tensor.matmul(
                            psq[:], wq_s[:, dc, ic * 128:(ic + 1) * 128],
                            xqT[:, dc, :], start=(dc == 0), stop=(dc == 7))
                    nc.scalar.copy(qt[:, ic, :], psq[:])

                # KT: [128 ic, 2048 j]
                for ic in range(8 if "phase1" not in ab else 0):
                    psk = ps1.tile([128, 4, 512], dt.float32, tag="psk")
                    for jc in range(4):
                        for dc in range(8):
                            nc.tensor.matmul(
                                psk[:, jc, :],
                                wk_s[:, dc, ic * 128:(ic + 1) * 128],
                                xT[:, dc, jc * 512:(jc + 1) * 512],
                                start=(dc == 0), stop=(dc == 7))
                    nc.scalar.copy(kt[:, ic, 128:2176], psk[:])

                # V: [128 j, 1024 kd] per 128-row chunk
                for jp in range(16 if "phase1" not in ab else 0):
                    psv = ps1.tile([128, 2, 512], dt.float32, tag="psv")
                    for dc in range(8):
                        for fh in range(2):
                            nc.tensor.matmul(
                                psv[:, fh, :],
                                xT[:, dc, jp * 128:(jp + 1) * 128],
                                wv_s[:, dc, fh * 512:(fh + 1) * 512],
                                start=(dc == 0), stop=(dc == 7))
                    nc.scalar.copy(v_sb[:, 1 + jp, :], psv[:])

            # ---------------- per-slot: QK + mix + attn + AV + Wo --------
            with tc.tile_pool(name="pmain", bufs=1) as pm:
              A = pm.tile([128, 16, 2176], dt.bfloat16)
              for s in range(NSLOT):
                J = SLOT_J[s]
                NJP = J // 128
                NC5 = (J + 511) // 512   # 512-wide chunks
                NC2 = (J + 767) // 768   # 768-wide dots chunks

                # ---- QK + talking-heads pre-mix -> A (dm) ----
                with tc.tile_pool(name="qkm", bufs=1) as pq, \
                     tc.tile_pool(name="shufp", bufs=1) as pshuf, \
                     tc.tile_pool(name="psQK", bufs=1, space="PSUM") as psqk, \
                     tc.tile_pool(name="psMix", bufs=1, space="PSUM") as psmx:
                    for c2 in range(NC2):
                        j0 = c2 * 768
                        W2 = min(768, J - j0)
                        ncs = (W2 + 511) // 512
                        dots_c = pq.tile([128, 16, 768], dt.bfloat16,
                                         tag="dots")
                        if "qk" in ab:
                            nc.vector.memset(dots_c[:], 0.01)
                        for cs in range(ncs if "qk" not in ab else 0):
                            W5 = min(512, W2 - cs * 512)
                            for h4 in range(4):
                                psd = psqk.tile([128, 4, 512], dt.float32,
                                                tag="psd")
                                for hh in range(4):
                                    h = h4 * 4 + hh
                                    hp = (h % 2) * 64
                                    nc.tensor.matmul(
                                        psd[:, hh, 0:W5],
                                        qt[hp:hp + 64, h // 2,
                                           s * 128:(s + 1) * 128],
                                        kt[hp:hp + 64, h // 2,
                                           j0 + cs * 512:j0 + cs * 512 + W5],
                                        start=True, stop=True)
                                nc.scalar.copy(
                                    dots_c[:, h4 * 4:h4 * 4 + 4,
                                           cs * 512:cs * 512 + W5],
                                    psd[:, :, 0:W5])
                        if "mix" in ab:
                            nc.scalar.copy(A[:, :, j0:j0 + W2],
                                           dots_c[:, :, 0:W2])
                        for gq in range(4 if "mix" not in ab else 0):
                            shufs = []
                            for gi in range(4):
                                g = gq * 4 + gi
                                shuf = pshuf.tile([128, 768], dt.bfloat16,
                                                  tag=f"shuf{gi}",
                                                  name=f"shuf{gi}")
                                nc.gpsimd.dma_start(
                                    shuf[:, 0:W2],
                                    dots_c[g * 8:(g + 1) * 8, :, 0:W2])
                                shufs.append(shuf)
                            for cs in range(ncs):
                                W5 = min(512, W2 - cs * 512)
                                psm = psmx.tile([128, 4, 512], dt.float32,
                                                tag="psm")
                                for gi in range(4):
                                    nc.tensor.matmul(
                                        psm[:, gi, 0:W5], w1_s[:],
                                        shufs[gi][:, cs * 512:cs * 512 + W5],
                                        start=True, stop=True)
                                nc.scalar.copy(
                                    A[:, gq * 4:gq * 4 + 4,
                                      j0 + cs * 512:j0 + cs * 512 + W5],
                                    psm[:, :, 0:W5])

                # ---- causal masks + pad mask ----
                nc.vector.memset(A[:, :, 16:128], NEG)
                for g in range(16):
                    woff = J - 640 + 8 * g
                    nc.gpsimd.tensor_add(
                        A[:, g, woff:J], A[:, g, woff:J],
                        masks_s[:, s, 0:640 - 8 * g])

                # ---- per row-group: top-64 + softmax -> p~ (in place) ----
                with tc.tile_pool(name="att", bufs=1) as pa, \
                     tc.tile_pool(name="psP", bufs=1, space="PSUM") as psp:
                    for g in range(16):
                        dm = A[:, g, 0:J]
                        scr = pa.tile([128, 2176], dt.bfloat16, tag="scr")
                        cands = pa.tile([128, 64], dt.float32, tag="cands")
                        if "topk" in ab:
                            nc.vector.memset(cands[:], 0.0)
                        else:
                            nc.vector.max(cands[:, 0:8], dm)
                            nc.vector.match_replace(scr[:, 0:J],
                                                    cands[:, 0:8], dm, NEG)
                            for r in range(1, 8):
                                nc.vector.max(cands[:, r * 8:(r + 1) * 8],
                                              scr[:, 0:J])
                                if r < 7:
                                    nc.vector.match_replace(
                                        scr[:, 0:J],
                                        cands[:, r * 8:(r + 1) * 8],
                                        scr[:, 0:J], NEG)
                        negm = pa.tile([128, 1], dt.float32, tag="negm")
                        nc.vector.tensor_scalar_mul(negm[:], cands[:, 0:1],
                                                    -1.0)
                        p2 = pa.tile([128, 2176], dt.bfloat16, tag="p2")
                        nc.scalar.activation(p2[:, 0:J], dm, AF.Exp,
                                             bias=negm[:])
                        den = pa.tile([128, 1], dt.float32, tag="den")
                        nc.vector.scalar_tensor_tensor(
                            dm, dm, cands[:, 63:64], p2[:, 0:J],
                            op0=ALU.is_ge, op1=ALU.mult, accum_out=den[:])
                        rden = pa.tile([128, 1], dt.float32, tag="rden")
                        nc.vector.reciprocal(rden[:], den[:])
                        nc.vector.tensor_scalar_mul(dm, dm, rden[:])

                        # post-mix (w2): p~ -> p^ written back into A
                        nps = (NC5 + 3) // 4
                        for q4 in range(nps if "postmix" not in ab else 0):
                            pspost = psp.tile([128, 4, 512], dt.float32,
                                              tag="pspost")
                            nq = min(4, NC5 - q4 * 4)
                            for qq in range(nq):
                                c5 = q4 * 4 + qq
                                W5 = min(512, J - c5 * 512)
                                nc.tensor.matmul(
                                    pspost[:, qq, 0:W5], w2_s[:],
                                    A[:, g, c5 * 512:c5 * 512 + W5],
                                    start=True, stop=True)
                            W4 = min(2048, J - q4 * 2048)
                            nc.vector.tensor_copy(
                                A[:, g, q4 * 2048:q4 * 2048 + W4],
                                pspost.rearrange("p a b -> p (a b)")[:, 0:W4])

                    # ---- AV ----
                    psav = psp.tile([128, 8, 128], dt.float32, tag="psav")
                    if "av" in ab:
                        nc.vector.memset(psav[:], 0.01)
                    for k in range(16 if "av" not in ab else 0):
                        pkh = pa.tile([128, 2176], dt.bfloat16, tag="pkh")
                        nc.gpsimd.dma_start(
                            pkh[:, 0:J], A[k * 8:(k + 1) * 8, :, 0:J])
                        pTk = pa.tile([128, 17, 128], dt.bfloat16, tag="pTk")
                        if "transpose" not in ab:
                            nc.sync.dma_start_transpose(
                                pTk[:, 0:NJP, :], pkh[:, 0:J])
                        else:
                            nc.vector.tensor_copy(
                                pTk[:, 0:NJP, :],
                                pkh[:, 0:J].rearrange("p (a b) -> p a b",
                                                      b=128))
                        kp = (k % 2) * 64
                        for jp in range(NJP):
                            nc.tensor.matmul(
                                psav[kp:kp + 64, k // 2, :],
                                v_sb[:, jp, k * 64:(k + 1) * 64],
                                pTk[:, jp, :],
                                start=(jp == 0), stop=(jp == NJP - 1))
                    av = pa.tile([128, 8, 128], dt.bfloat16, tag="av")
                    nc.vector.tensor_copy(av[:], psav[:])

                    # ---- Wo projection + bias ----
                    pso = psp.tile([128, 2, 512], dt.float32, tag="pso")
                    for kk in range(8):
                        for fh in range(2):
                            nc.tensor.matmul(
                                pso[:, fh, :], av[:, kk, :],
                                wo_s[:, kk, fh * 512:(fh + 1) * 512],
                                start=(kk == 0), stop=(kk == 7))
                    osb = pa.tile([128, DIM], dt.float32, tag="osb")
                    nc.vector.tensor_add(
                        osb[:], pso.rearrange("p a b -> p (a b)")[:], bo_s[:])
                    nc.sync.dma_start(out_d[s, :, :], osb[:])

    nc.finalize()
    return nc


def _host_prep(core, inputs):
    x = np.asarray(inputs["x"], dtype=np.float32)
    Wq = np.asarray(inputs["Wq"], dtype=np.float32) * (D ** -0.5)
    Wk = np.asarray(inputs["Wk"], dtype=np.float32)
    Wv = np.asarray(inputs["Wv"], dtype=np.float32)
    Wo = np.asarray(inputs["Wo"], dtype=np.float32)
    bo = np.asarray(inputs["bo"], dtype=np.float32)
    pre = np.asarray(inputs["pre_proj"], dtype=np.float32)
    post = np.asarray(inputs["post_proj"], dtype=np.float32)
    mem_k = np.asarray(inputs["mem_k"], dtype=np.float32)
    mem_v = np.asarray(inputs["mem_v"], dtype=np.float32)

    b = core // 4
    g4 = core % 4
    qbs = QBS_OF_G[g4]

    xb = x[b]
    if ("xT", b) not in _prep_cache:
        xT = np.ascontiguousarray(xb.T).astype(bf16)  # [DIM, N]
        _prep_cache[("xT", b)] = np.ascontiguousarray(
            xT.reshape(8, 128, N).transpose(1, 0, 2))
    xT_c = _prep_cache[("xT", b)]

    xq = np.concatenate([xb[qb * 128:(qb + 1) * 128] for qb in qbs], axis=0)
    xqT = np.ascontiguousarray(xq.T).astype(bf16)     # [DIM, 512]
    xqT_c = np.ascontiguousarray(xqT.reshape(8, 128, 512).transpose(1, 0, 2))

    def chunk_w(w):
        return np.ascontiguousarray(
            w.astype(bf16).reshape(8, 128, DIM).transpose(1, 0, 2))

    if "w" not in _prep_cache:
        wo_r = np.zeros((128, 8, DIM), dtype=np.float32)
        for k in range(H):
            wo_r[(k % 2) * 64:(k % 2) * 64 + 64, k // 2, :] = \
                Wo[k * 64:(k + 1) * 64, :]

        mkt = np.zeros((128, 8, 128), dtype=np.float32)
        for h in range(H):
            # kt[p, ic, j] = K[j, ic*128+p]; mem key j<16, inner c = h*64+d
            for dd in range(D):
                c = h * 64 + dd
                mkt[c % 128, c // 128, 0:M] = mem_k[h, :, dd]
        mv = np.zeros((128, DIM), dtype=np.float32)
        mv[0:M] = mem_v.transpose(1, 0, 2).reshape(M, DIM)

        w1 = np.zeros((128, 128), dtype=np.float32)
        w2 = np.zeros((128, 128), dtype=np.float32)
        for i in range(8):
            for h in range(H):
                for k in range(H):
                    w1[i * 16 + h, i * 16 + k] = pre[h, k]
                    w2[i * 16 + h, k * 8 + i] = post[h, k]

        _prep_cache["w"] = (
            chunk_w(Wq), chunk_w(Wk), chunk_w(Wv),
            wo_r.astype(bf16), mkt.astype(bf16), mv.astype(bf16),
            w1.astype(bf16), w2.astype(bf16),
            np.ascontiguousarray(
                np.broadcast_to(bo[None, :], (128, DIM)).copy()))
    (wq_c, wk_c, wv_c, wo_r, mkt, mv, w1, w2, bo_b) = _prep_cache["w"]

    masks = np.zeros((128, NSLOT, 640), dtype=np.float32)
    for si, qb in enumerate(qbs):
        qbmax = max(QBS_OF_G[gg][si] for gg in range(4))
        brel_base = 513 - 128 * (qbmax - qb)
        cols = np.arange(640)
        for p in range(128):
            i = p // 16
            masks[p, si, cols >= brel_base + i] = NEG

    return {
        "xT": xT_c, "xqT": xqT_c,
        "wq": wq_c, "wk": wk_c, "wv": wv_c, "wo": wo_r,
        "memKT": mkt, "memV": mv,
        "w1": w1, "w2": w2,
        "masks": masks.astype(bf16),
        "bo": bo_b,
    }


def kernel(**inputs) -> np.ndarray:
    from concourse.bass_utils import run_bass_kernel_spmd

    _prep_cache.clear()

    if "nc" not in _nc_cache:
        _nc_cache["nc"] = build_nc()
    nc = _nc_cache["nc"]

    in_maps = [_host_prep(c, inputs) for c in range(8)]
    res = run_bass_kernel_spmd(nc, in_maps, core_ids=list(range(8)))

    rr = np.arange(128)
    perm = (rr % 16) * 8 + rr // 16   # row r=(i*16+g) holds query g*8+i
    out = np.zeros((B, N, DIM), dtype=np.float32)
    for c in range(8):
        b = c // 4
        qbs = QBS_OF_G[c % 4]
        o = res.results[c]["out"]
        for si, qb in enumerate(qbs):
            out[b, qb * 128 + perm, :] = o[si]
    return out
